# revision 8
# baseline (speedup 1.0000x reference)
"""GCNConv + PReLU on Trainium2, 8-core SPMD Bass/Tile kernel (v3).

Math (PyG GCNConv, add_self_loops=True, symmetric norm):
    h = x @ W
    deg[c] = (# edges with col == c) + 1          (self-loop)
    dis = rsqrt(deg)
    out[c] = dis[c] * ( sum_{e: col_e == c} dis[row_e] * h[row_e]
                        + dis[c] * h[c] )         (self-loop folded in)
             + bias
    z = max(out, prelu_a * out)                   (PReLU, 0 <= a <= 1)

v3 redesign vs v2 (which computed the full g table on every core):
  - Node rows are split into 8 blocks of BLK=12544 (=98*128) rows; each
    core computes g = dis*h for ITS block only (8x less PE+DMA in phase
    B), then an AllGather shares the 4-sub-table g layout to all cores.
    Dest shard == block, so the self-loop term dis*h lands in the accw
    accumulator during the same pass (phase Bown is gone).
  - The per-edge dma_gather is the serial bottleneck: its SWDGE
    descriptor GENERATION costs ~4ns/edge on the one GpSimd Q7 pair
    (~16us per 4096-edge chunk), while the DMA transfer itself
    overlaps. v3 issues all gathers as prepare_only preps (descriptor
    gen only depends on the host-built index tensor, so the Pool engine
    streams gen from t~0, under phase B), and fires each chunk with
    trigger_dma(count=1) once the AllGather output and the chunk's SBUF
    ring buffer are ready. Tile moves the data deps from prep to
    trigger automatically.

Device pipeline per core:
  B') own-block h = x@W tile-by-tile in bf16, row-scaled by dis into
     the AG input table (partition-major, 2KB lines) and into accw
     (f32, the self-loop term). Then AllGather -> full g table in DRAM.
  C) per 128-edge tile: bulk dma_gather (prep early / trigger when
     ready) of source g rows + PE matmul onehotT @ gathered, window
     sums accumulated in SBUF accw across the 4 sub-table rounds;
     flush = dis-scale + bias + PReLU and DMA out.
"""

import math
import sys

for _p in ("/opt/trn_rl_repo",):
    if _p not in sys.path:
        sys.path.insert(0, _p)

import numpy as np
import ml_dtypes

P = 128
CORES = 8
NSUB = 4
GCT = 32  # tiles per dma_gather chunk
XCH = 1024  # x columns per load
GB = 8  # g row-tiles staged per DMA write (= XCH // P)
WARM = 8  # gather preps issued before phase B (pipeline warmup)
GBUFS = 5  # gathered-chunk SBUF ring depth

FULL_CFG = dict(N=100000, F_IN=256, F_OUT=128, E=1600000)

_prog_cache = {}


def _derived(cfg):
    N = cfg["N"]
    NT = math.ceil(N / P)  # global row-tiles
    NT = -(-NT // CORES) * CORES  # pad so blocks split evenly
    NP = NT * P
    bt = NT // CORES  # row-tiles per block
    BLK = bt * P  # rows per block (dest shard == source block)
    # sub-table s = blocks 2s, 2s+1 of the AllGather output; row count
    # per sub-table must stay under the int16 gather-index limit.
    assert 2 * BLK <= 32767
    return NT, NP, bt, BLK


def _schedule(tsw, nw):
    """Segment layout: one contiguous tile run per sub-table s
    (w-ascending within), padded to a GCT multiple. Returns per-(w,s)
    tile base offsets, segment table, and total padded tile count T."""
    tile_base = np.zeros((nw, NSUB), np.int64)
    segs = []  # (start, real, padded) per s
    pos = 0
    for s in range(NSUB):
        start = pos
        for w in range(nw):
            tile_base[w, s] = pos
            pos += tsw[w][s]
        real = pos - start
        padded = -(-real // GCT) * GCT
        segs.append((start, real, padded))
        pos = start + padded
    return tile_base, segs, pos


def host_prep(x, edge_index, W, bias, prelu_a, cfg):
    """Index/layout prep: integer degree histogram, shard + sort edges by
    (dest window, source sub-table), fixed 128-edge tiles, int16 wrapped
    gather indices (partition-major g layout), fp8 one-hot tiles, and
    bf16 cast/transpose of x. All float math on x/W stays on device."""
    N, F_IN, F_OUT = cfg["N"], cfg["F_IN"], cfg["F_OUT"]
    NT, NP, bt, BLK = _derived(cfg)
    nw = bt  # dest windows per core == block row-tiles

    x = np.asarray(x, np.float32)
    W = np.asarray(W, np.float32)
    bias = np.asarray(bias, np.float32)
    prelu_a = np.asarray(prelu_a, np.float32)
    ei = np.asarray(edge_index)

    rows_all = np.asarray(ei[0]).astype(np.int64)
    cols_all = np.asarray(ei[1]).astype(np.int64)

    # integer degree histogram (+1 self-loop); pad rows get deg=1
    deg = np.bincount(cols_all, minlength=N).astype(np.float32) + 1.0
    deg_pad = np.ones(NP, np.float32)
    deg_pad[:N] = deg
    # per-core block layout [CORES, P, bt]: deg_blk[k, p, j] = deg of
    # global row k*BLK + j*P + p
    deg_blk = np.ascontiguousarray(
        deg_pad.reshape(CORES, bt, P).transpose(0, 2, 1)
    )

    order = np.argsort(cols_all, kind="stable")
    rs = rows_all[order]
    cs = cols_all[order]
    bounds = np.searchsorted(cs, np.arange(CORES + 1) * BLK)

    # source row r -> gather position: block k = j//bt (j = r//P), then
    # partition-major within block: pos = k*BLK + (r%P)*bt + j%bt.
    # Sub-table s = k//2, in-table index = pos - s*2*BLK.
    cnts = np.zeros((CORES, nw, NSUB), np.int64)
    per_core = []
    for k in range(CORES):
        seg = slice(bounds[k], bounds[k + 1])
        local = cs[seg] - k * BLK
        w_arr = local // P
        r_arr = rs[seg]
        j_arr = r_arr // P
        s_arr = j_arr // (2 * bt)
        key = w_arr * NSUB + s_arr
        o2 = np.argsort(key, kind="stable")
        cnts[k] = np.bincount(key, minlength=nw * NSUB).reshape(nw, NSUB)
        per_core.append((local[o2], w_arr[o2], s_arr[o2], r_arr[o2], key[o2]))

    tsw = (-(-cnts // P)).max(axis=0)  # [nw, NSUB]
    tsw[:, 0] = np.maximum(tsw[:, 0], 1)  # every window needs >= 1 matmul
    tsw_l = tuple(tuple(int(v) for v in row) for row in tsw)
    tile_base, segs, T = _schedule(tsw_l, nw)

    idx16 = np.zeros((CORES, 16, T * 8), np.int16)
    oh8 = np.zeros((CORES, P, T * P), ml_dtypes.float8_e4m3)
    for k in range(CORES):
        local, w_arr, s_arr, r_arr, key = per_core[k]
        cnt_flat = cnts[k].reshape(-1)
        gstart = np.concatenate([[0], np.cumsum(cnt_flat)])
        within = np.arange(local.size) - gstart[key]
        slot = tile_base[w_arr, s_arr] * P + within
        p_arr = slot % P
        t_arr = slot // P
        # gather position: partition-major within block, block-major in s
        kb = r_arr // P // bt
        pos = kb * BLK + (r_arr % P) * bt + (r_arr // P) % bt
        pos -= s_arr * (2 * BLK)
        idx16[k, p_arr % 16, t_arr * 8 + p_arr // 16] = pos.astype(np.int16)
        oh8[k, p_arr, t_arr * P + (local - w_arr * P)] = 1.0
        # trailing pad tiles of each segment: idx = -1 (descriptors skipped)
        for start, real, padded in segs:
            if padded > real:
                idx16[k, :, (start + real) * 8 : (start + padded) * 8] = -1
    idx16_rep = np.ascontiguousarray(np.tile(idx16, (1, P // 16, 1)))

    xp = np.zeros((NP, F_IN), np.float32)
    xp[:N] = x
    x_t = np.ascontiguousarray(xp.T.astype(ml_dtypes.bfloat16))  # [F_IN, NP]
    # per-core block slice of x (transposed): [CORES, F_IN, BLK]
    x_blk = np.ascontiguousarray(x_t.reshape(F_IN, CORES, BLK).transpose(1, 0, 2))

    return dict(
        tsw=tsw_l,
        T=T,
        x_blk=x_blk,
        w=np.ascontiguousarray(W.astype(ml_dtypes.bfloat16)),
        bias_b=np.ascontiguousarray(np.tile(bias[None, :], (P, 1))),
        prelu_b=np.ascontiguousarray(np.tile(prelu_a[None, :], (P, 1))),
        deg_blk=deg_blk,
        idx16=idx16_rep,
        oh8=oh8,
    )


def build_program(cfg, tsw, debug_outs=False):
    import concourse.bass as bass
    import concourse.bacc as bacc
    import concourse.mybir as mybir
    import concourse.tile as tile
    from concourse.bass import ds

    f32 = mybir.dt.float32
    bf16 = mybir.dt.bfloat16
    fp8 = mybir.dt.float8e4
    i16 = mybir.dt.int16
    AOT = mybir.AluOpType
    ACT = mybir.ActivationFunctionType

    N, F_IN, F_OUT = cfg["N"], cfg["F_IN"], cfg["F_OUT"]
    NT, NP, bt, BLK = _derived(cfg)
    nw = bt
    kchunks = F_IN // P
    tile_base, segs, T = _schedule(tsw, nw)

    nc = bacc.Bacc(
        "TRN2",
        target_bir_lowering=False,
        debug=False,
        num_devices=CORES,
        num_swdge_queues=4,
    )

    x_blk = nc.dram_tensor("x_blk", [F_IN, BLK], bf16, kind="ExternalInput")
    w_d = nc.dram_tensor("w", [F_IN, F_OUT], bf16, kind="ExternalInput")
    bias_d = nc.dram_tensor("bias_b", [P, F_OUT], f32, kind="ExternalInput")
    prelu_d = nc.dram_tensor("prelu_b", [P, F_OUT], f32, kind="ExternalInput")
    degb_d = nc.dram_tensor("deg_blk", [P, nw], f32, kind="ExternalInput")
    idx16_d = nc.dram_tensor("idx16", [P, T * 8], i16, kind="ExternalInput")
    oh_d = nc.dram_tensor("oh8", [P, T * P], fp8, kind="ExternalInput")
    out_d = nc.dram_tensor("out", [BLK, F_OUT], f32, kind="ExternalOutput")

    # AllGather: per-core g block (partition-major) -> full table, of
    # which the 4 gather sub-tables are row-range views.
    ag_in = nc.dram_tensor("ag_in", [BLK, F_OUT], bf16)
    ag_out = nc.dram_tensor("ag_out", [CORES * BLK, F_OUT], bf16)
    SUBR = 2 * BLK  # rows per sub-table view

    with tile.TileContext(nc, pool_alloc_mode="queue") as tc:
        with (
            tc.tile_pool(name="const", bufs=1) as constp,
            tc.tile_pool(name="dis", bufs=1) as disp,
            tc.tile_pool(name="acc", bufs=1) as accp,
            tc.tile_pool(name="c_oh", bufs=4) as cohp,
            tc.tile_pool(name="c_g", bufs=GBUFS) as cgp,
            tc.tile_pool(name="c_ps", bufs=8, space="PSUM") as cpsp,
            tc.tile_pool(name="c_f", bufs=4) as cfp,
            tc.tile_pool(name="b_x", bufs=3) as bxp,
            tc.tile_pool(name="b_g", bufs=4) as bgp,
        ):
            wt = []
            for c in range(kchunks):
                wc = constp.tile([P, F_OUT], bf16, tag=f"wc{c}")
                nc.sync.dma_start(out=wc[:], in_=w_d[c * P : (c + 1) * P, :])
                wt.append(wc)
            biasb = constp.tile([P, F_OUT], f32)
            nc.sync.dma_start(out=biasb[:], in_=bias_d[:, :])
            prelub = constp.tile([P, F_OUT], f32)
            nc.sync.dma_start(out=prelub[:], in_=prelu_d[:, :])

            # idx array resident in SBUF, loaded in 4 segment pieces so
            # the first gather preps can start almost immediately
            idx_all = disp.tile([P, T * 8], i16, name="idx_all")
            for s in range(NSUB):
                start, real, padded = segs[s]
                end = start + padded
                nc.sync.dma_start(
                    out=idx_all[:, start * 8 : end * 8],
                    in_=idx16_d[:, start * 8 : end * 8],
                )

            dis_b = disp.tile([P, nw], f32)
            nc.sync.dma_start(out=dis_b[:], in_=degb_d[:, :])
            nc.scalar.activation(out=dis_b[:], in_=dis_b[:], func=ACT.Sqrt)
            nc.vector.reciprocal(out=dis_b[:], in_=dis_b[:])

            accw = accp.tile([P, nw * F_OUT], f32, name="accw")

            # ------------- phase C gather preps (descriptor gen) ---------
            # chunk table: (sub-table, first tile, real tiles)
            chunks = []
            for s in range(NSUB):
                start, real, padded = segs[s]
                for ck0 in range(0, real, GCT):
                    chunks.append((s, start + ck0, min(real - ck0, GCT)))
            NCH = len(chunks)

            # ---------------- Phase B': g block = dis * (x @ W) ----------
            # h for the core's own 12544 rows; scaled rows go to the
            # AllGather input (bf16) AND seed accw with the self-loop
            # term dis*h (f32). Dest windows align with block tiles.
            with nc.named_scope("phaseB"):
                chunks_b = list(range(0, BLK, XCH))
                LA = 2
                xtiles = {}

                def load_x(ci):
                    if ci >= len(chunks_b):
                        return
                    c0 = chunks_b[ci]
                    cl = min(XCH, BLK - c0)
                    xt = bxp.tile([P, 2 * XCH], bf16, tag="xt", name="xt")
                    nc.scalar.dma_start(
                        out=xt[:, : 2 * cl].rearrange("p (t c) -> p t c", t=2),
                        in_=x_blk[:, c0 : c0 + cl].rearrange(
                            "(t p) c -> p t c", p=P
                        ),
                    )
                    xtiles[ci] = xt

                for ci in range(LA):
                    load_x(ci)
                for ci, c0 in enumerate(chunks_b):
                    cl = min(XCH, BLK - c0)
                    load_x(ci + LA)
                    xt = xtiles.pop(ci)
                    gt = bgp.tile([P, GB * F_OUT], bf16, tag="bg")
                    jt0 = c0 // P
                    ntiles = cl // P
                    assert ntiles <= GB
                    for jj in range(ntiles):
                        j = jt0 + jj
                        ph = cpsp.tile([P, F_OUT], f32, tag="cps", name="cps")
                        for c in range(kchunks):
                            nc.tensor.matmul(
                                out=ph[:],
                                lhsT=xt[:, c * cl + jj * P : c * cl + (jj + 1) * P],
                                rhs=wt[c][:],
                                start=(c == 0),
                                stop=(c == kchunks - 1),
                            )
                        # g row = dis * h (Scalar engine), AG input
                        nc.scalar.mul(
                            gt[:, jj * F_OUT : (jj + 1) * F_OUT],
                            ph[:],
                            dis_b[:, j : j + 1],
                        )
                        # self-loop seed: accw = dis * h (DVE, f32)
                        nc.vector.tensor_scalar(
                            out=accw[:, j * F_OUT : (j + 1) * F_OUT],
                            in0=ph[:],
                            scalar1=dis_b[:, j : j + 1],
                            scalar2=None,
                            op0=AOT.mult,
                        )
                    # partition-major block write: row 128j+p at p*bt+j
                    nc.scalar.dma_start(
                        out=ag_in.rearrange("(p j) f -> p j f", p=P)[
                            :, jt0 : jt0 + ntiles, :
                        ],
                        in_=gt[:, : ntiles * F_OUT].rearrange(
                            "p (j f) -> p j f", f=F_OUT
                        ),
                    )

                nc.gpsimd.collective_compute(
                    "AllGather",
                    mybir.AluOpType.bypass,
                    replica_groups=[list(range(CORES))],
                    ins=[ag_in[:, :]],
                    outs=[ag_out[:, :]],
                )

            # ---------------- Phase C: trigger gathers + scatter matmuls -
            with nc.named_scope("phaseC"):
                slast = [
                    max(s for s in range(NSUB) if (tsw[w][s] > 0 or s == 0))
                    for w in range(nw)
                ]

                def flush(w):
                    acc = cfp.tile([P, F_OUT], f32, tag="facc", name="facc")
                    nc.scalar.mul(
                        acc[:],
                        accw[:, w * F_OUT : (w + 1) * F_OUT],
                        dis_b[:, w : w + 1],
                    )
                    nc.vector.tensor_tensor(
                        out=acc[:], in0=acc[:], in1=biasb[:], op=AOT.add
                    )
                    neg = cfp.tile([P, F_OUT], f32, tag="fneg", name="fneg")
                    nc.vector.tensor_tensor(
                        out=neg[:], in0=acc[:], in1=prelub[:], op=AOT.mult
                    )
                    nc.vector.tensor_tensor(
                        out=acc[:], in0=acc[:], in1=neg[:], op=AOT.max
                    )
                    nc.scalar.dma_start(
                        out=out_d[w * P : (w + 1) * P, :], in_=acc[:, :]
                    )

                ci = 0
                for s in range(NSUB):
                    start, real, padded = segs[s]
                    tmap = []
                    bfirst = []
                    blast = []
                    for w in range(nw):
                        nt = tsw[w][s]
                        tmap += [w] * nt
                        bfirst += [True] + [False] * (nt - 1) if nt else []
                        blast += [False] * (nt - 1) + [True] if nt else []
                    cur = -1
                    oht = gch = None
                    ps = None
                    for tg in range(real):
                        ck = tg // GCT
                        if ck != cur:
                            cur = ck
                            s_c, t0, nreal = chunks[ci]
                            assert s_c == s and t0 == start + ck * GCT
                            gch = cgp.tile([P, GCT * F_OUT], bf16, tag="cg", name="cg")
                            nc.gpsimd.dma_gather(
                                out_ap=gch[:].rearrange("p (n e) -> p n e", e=F_OUT),
                                in_ap=ag_out[s * SUBR : (s + 1) * SUBR, :],
                                idxs_ap=idx_all[:, t0 * 8 : (t0 + GCT) * 8],
                                num_idxs=GCT * P,
                                num_idxs_reg=nreal * P,
                                elem_size=F_OUT,
                                single_packet=False,
                                queue_num=ci % 4,
                            )
                            oht = cohp.tile([P, GCT * P], fp8, tag="oh", name="oh")
                            nc.sync.dma_start(
                                out=oht[:, : nreal * P],
                                in_=oh_d[:, t0 * P : (t0 + nreal) * P],
                            )
                            ci += 1
                        ti = tg - cur * GCT
                        w = tmap[tg]
                        if bfirst[tg]:
                            ps = cpsp.tile([P, F_OUT], f32, tag="cps", name="cps")
                        nc.tensor.matmul(
                            out=ps[:],
                            lhsT=oht[:, ti * P : (ti + 1) * P],
                            rhs=gch[:, ti * F_OUT : (ti + 1) * F_OUT],
                            start=bfirst[tg],
                            stop=blast[tg],
                        )
                        if blast[tg]:
                            nc.vector.tensor_tensor(
                                out=accw[:, w * F_OUT : (w + 1) * F_OUT],
                                in0=accw[:, w * F_OUT : (w + 1) * F_OUT],
                                in1=ps[:],
                                op=AOT.add,
                            )
                            if s == slast[w]:
                                flush(w)
                assert ci == NCH

    nc.compile()
    return nc


def _get_program(cfg, tsw, debug_outs=False):
    key = (tuple(sorted(cfg.items())), tsw, debug_outs)
    if key not in _prog_cache:
        _prog_cache[key] = build_program(cfg, tsw, debug_outs)
    return _prog_cache[key]


def make_in_maps(prep):
    return [
        {
            "x_blk": prep["x_blk"][k],
            "w": prep["w"],
            "bias_b": prep["bias_b"],
            "prelu_b": prep["prelu_b"],
            "deg_blk": prep["deg_blk"][k],
            "idx16": prep["idx16"][k],
            "oh8": prep["oh8"][k],
        }
        for k in range(CORES)
    ]


def kernel(x, edge_index, W, bias, prelu_a, cfg=None):
    from concourse import bass_utils

    cfg = cfg or FULL_CFG
    cfg = dict(cfg)
    prep = host_prep(x, edge_index, W, bias, prelu_a, cfg)
    nc = _get_program(cfg, prep["tsw"])
    res = bass_utils.run_bass_kernel_spmd(
        nc, make_in_maps(prep), core_ids=list(range(CORES))
    )
    N = cfg["N"]
    NT, NP, bt, BLK = _derived(cfg)
    outs = []
    for k in range(CORES):
        lo = k * BLK
        hi = min((k + 1) * BLK, N)
        outs.append(res.results[k]["out"][: hi - lo])
    return np.concatenate(outs, axis=0).astype(np.float32)


# revision 17
# speedup vs baseline: 1.1843x; 1.1843x over previous
"""GCNConv + PReLU on Trainium2, 8-core SPMD Bass/Tile kernel (v5).

Math (PyG GCNConv, add_self_loops=True, symmetric norm):
    h = x @ W
    deg[c] = (# edges with col == c) + 1          (self-loop)
    dis = rsqrt(deg)
    out[c] = dis[c] * ( sum_{e: col_e == c} dis[row_e] * h[row_e]
                        + dis[c] * h[c] )         (self-loop folded in)
             + bias
    z = max(out, prelu_a * out)                   (PReLU, 0 <= a <= 1)

Distribution (v3+): node rows split into 8 blocks of BLK=12544 rows;
each core computes g = dis*h for its block only, AllGather shares the
table; dest shard == source block so the self-loop term seeds the accw
accumulator during the same pass.

v5 phase C: per-edge dma_gather of source g rows from the DRAM table.
Trace analysis showed the binding constraint is the SDMA per-descriptor
processing wall (~100ns/desc/engine) when each 256B descriptor is its
own packet (single_packet=False). v5 uses single_packet=True with
1024-index chunks (64 descriptors per engine = the HW packet-coalescing
limit), which lets each engine stream its descriptors back-to-back.
"""

import math
import sys

for _p in ("/opt/trn_rl_repo",):
    if _p not in sys.path:
        sys.path.insert(0, _p)

import numpy as np
import ml_dtypes

P = 128
CORES = 8
NSUB = 4
GCT = 8  # tiles per dma_gather chunk (64 descs/engine: single-packet max)
XCH = 1024  # x columns per load
GB = 8  # g row-tiles staged per DMA write (= XCH // P)
GBUFS = 12  # gathered-chunk SBUF ring depth

FULL_CFG = dict(N=100000, F_IN=256, F_OUT=128, E=1600000)

_prog_cache = {}


def _derived(cfg):
    N = cfg["N"]
    NT = math.ceil(N / P)  # global row-tiles
    NT = -(-NT // CORES) * CORES  # pad so blocks split evenly
    NP = NT * P
    bt = NT // CORES  # row-tiles per block
    BLK = bt * P  # rows per block (dest shard == source block)
    # sub-table s = blocks 2s, 2s+1 of the AllGather output; row count
    # per sub-table must stay under the int16 gather-index limit.
    assert 2 * BLK <= 32767
    return NT, NP, bt, BLK


def _schedule(tsw, nw):
    """Segment layout: one contiguous tile run per sub-table s
    (w-ascending within), padded to a GCT multiple. Returns per-(w,s)
    tile base offsets, segment table, and total padded tile count T."""
    tile_base = np.zeros((nw, NSUB), np.int64)
    segs = []  # (start, real, padded) per s
    pos = 0
    for s in range(NSUB):
        start = pos
        for w in range(nw):
            tile_base[w, s] = pos
            pos += tsw[w][s]
        real = pos - start
        padded = -(-real // GCT) * GCT
        segs.append((start, real, padded))
        pos = start + padded
    return tile_base, segs, pos


def host_prep(x, edge_index, W, bias, prelu_a, cfg):
    """Index/layout prep: integer degree histogram, shard + sort edges by
    (dest window, source sub-table), fixed 128-edge tiles, int16 wrapped
    gather indices (partition-major g layout), fp8 one-hot tiles, and
    bf16 cast/transpose of x. All float math on x/W stays on device."""
    N, F_IN, F_OUT = cfg["N"], cfg["F_IN"], cfg["F_OUT"]
    NT, NP, bt, BLK = _derived(cfg)
    nw = bt  # dest windows per core == block row-tiles

    x = np.asarray(x, np.float32)
    W = np.asarray(W, np.float32)
    bias = np.asarray(bias, np.float32)
    prelu_a = np.asarray(prelu_a, np.float32)
    ei = np.asarray(edge_index)

    rows_all = np.asarray(ei[0]).astype(np.int64)
    cols_all = np.asarray(ei[1]).astype(np.int64)

    # integer degree histogram (+1 self-loop); pad rows get deg=1
    deg = np.bincount(cols_all, minlength=N).astype(np.float32) + 1.0
    deg_pad = np.ones(NP, np.float32)
    deg_pad[:N] = deg
    # per-core block layout [CORES, P, bt]: deg_blk[k, p, j] = deg of
    # global row k*BLK + j*P + p
    deg_blk = np.ascontiguousarray(
        deg_pad.reshape(CORES, bt, P).transpose(0, 2, 1)
    )

    order = np.argsort(cols_all, kind="stable")
    rs = rows_all[order]
    cs = cols_all[order]
    bounds = np.searchsorted(cs, np.arange(CORES + 1) * BLK)

    # source row r -> gather position: block k = j//bt (j = r//P), then
    # partition-major within block: pos = k*BLK + (r%P)*bt + j%bt.
    # Sub-table s = k//2, in-table index = pos - s*2*BLK.
    cnts = np.zeros((CORES, nw, NSUB), np.int64)
    per_core = []
    for k in range(CORES):
        seg = slice(bounds[k], bounds[k + 1])
        local = cs[seg] - k * BLK
        w_arr = local // P
        r_arr = rs[seg]
        j_arr = r_arr // P
        s_arr = j_arr // (2 * bt)
        key = w_arr * NSUB + s_arr
        o2 = np.argsort(key, kind="stable")
        cnts[k] = np.bincount(key, minlength=nw * NSUB).reshape(nw, NSUB)
        per_core.append((local[o2], w_arr[o2], s_arr[o2], r_arr[o2], key[o2]))

    tsw = (-(-cnts // P)).max(axis=0)  # [nw, NSUB]
    tsw[:, 0] = np.maximum(tsw[:, 0], 1)  # every window needs >= 1 matmul
    tsw_l = tuple(tuple(int(v) for v in row) for row in tsw)
    tile_base, segs, T = _schedule(tsw_l, nw)

    idx16 = np.zeros((CORES, 16, T * 8), np.int16)
    oh8 = np.zeros((CORES, P, T * P), ml_dtypes.float8_e4m3)
    for k in range(CORES):
        local, w_arr, s_arr, r_arr, key = per_core[k]
        cnt_flat = cnts[k].reshape(-1)
        gstart = np.concatenate([[0], np.cumsum(cnt_flat)])
        within = np.arange(local.size) - gstart[key]
        slot = tile_base[w_arr, s_arr] * P + within
        p_arr = slot % P
        t_arr = slot // P
        # gather position: partition-major within block, block-major in s
        kb = (r_arr // P // bt) % 2
        pos = kb * BLK + (r_arr % P) * bt + (r_arr // P) % bt
        idx16[k, p_arr % 16, t_arr * 8 + p_arr // 16] = pos.astype(np.int16)
        oh8[k, p_arr, t_arr * P + (local - w_arr * P)] = 1.0
        # trailing pad tiles of each segment: idx = -1 (descriptors skipped)
        for start, real, padded in segs:
            if padded > real:
                idx16[k, :, (start + real) * 8 : (start + padded) * 8] = -1
    idx16_rep = np.ascontiguousarray(np.tile(idx16, (1, P // 16, 1)))

    xp = np.zeros((NP, F_IN), np.float32)
    xp[:N] = x
    x_t = np.ascontiguousarray(xp.T.astype(ml_dtypes.bfloat16))  # [F_IN, NP]
    # per-core block slice of x (transposed): [CORES, F_IN, BLK]
    x_blk = np.ascontiguousarray(x_t.reshape(F_IN, CORES, BLK).transpose(1, 0, 2))

    return dict(
        tsw=tsw_l,
        T=T,
        x_blk=x_blk,
        w=np.ascontiguousarray(W.astype(ml_dtypes.bfloat16)),
        bias_b=np.ascontiguousarray(np.tile(bias[None, :], (P, 1))),
        prelu_b=np.ascontiguousarray(np.tile(prelu_a[None, :], (P, 1))),
        deg_blk=deg_blk,
        idx16=idx16_rep,
        oh8=oh8,
    )


def build_program(cfg, tsw, debug_outs=False):
    import concourse.bass as bass
    import concourse.bacc as bacc
    import concourse.mybir as mybir
    import concourse.tile as tile
    from concourse.bass import ds

    f32 = mybir.dt.float32
    bf16 = mybir.dt.bfloat16
    fp8 = mybir.dt.float8e4
    i16 = mybir.dt.int16
    AOT = mybir.AluOpType
    ACT = mybir.ActivationFunctionType

    N, F_IN, F_OUT = cfg["N"], cfg["F_IN"], cfg["F_OUT"]
    NT, NP, bt, BLK = _derived(cfg)
    nw = bt
    kchunks = F_IN // P
    tile_base, segs, T = _schedule(tsw, nw)

    nc = bacc.Bacc(
        "TRN2",
        target_bir_lowering=False,
        debug=False,
        num_devices=CORES,
        num_swdge_queues=4,
    )

    x_blk = nc.dram_tensor("x_blk", [F_IN, BLK], bf16, kind="ExternalInput")
    w_d = nc.dram_tensor("w", [F_IN, F_OUT], bf16, kind="ExternalInput")
    bias_d = nc.dram_tensor("bias_b", [P, F_OUT], f32, kind="ExternalInput")
    prelu_d = nc.dram_tensor("prelu_b", [P, F_OUT], f32, kind="ExternalInput")
    degb_d = nc.dram_tensor("deg_blk", [P, nw], f32, kind="ExternalInput")
    idx16_d = nc.dram_tensor("idx16", [P, T * 8], i16, kind="ExternalInput")
    oh_d = nc.dram_tensor("oh8", [P, T * P], fp8, kind="ExternalInput")
    out_d = nc.dram_tensor("out", [BLK, F_OUT], f32, kind="ExternalOutput")

    # AllGather: per-core g block (partition-major) -> full table, of
    # which the 4 gather sub-tables are row-range views.
    ag_in = nc.dram_tensor("ag_in", [BLK, F_OUT], bf16)
    ag_out = nc.dram_tensor(
        "ag_out", [CORES * BLK, F_OUT], bf16, addr_space="Shared"
    )
    SUBR = 2 * BLK  # rows per sub-table view

    with tile.TileContext(nc, pool_alloc_mode="queue") as tc:
        with (
            tc.tile_pool(name="const", bufs=1) as constp,
            tc.tile_pool(name="dis", bufs=1) as disp,
            tc.tile_pool(name="acc", bufs=1) as accp,
            tc.tile_pool(name="c_oh", bufs=6) as cohp,
            tc.tile_pool(name="c_g", bufs=GBUFS) as cgp,
            tc.tile_pool(name="c_ps", bufs=8, space="PSUM") as cpsp,
            tc.tile_pool(name="c_f", bufs=4) as cfp,
            tc.tile_pool(name="b_x", bufs=3) as bxp,
            tc.tile_pool(name="b_g", bufs=4) as bgp,
        ):
            wt = []
            for c in range(kchunks):
                wc = constp.tile([P, F_OUT], bf16, tag=f"wc{c}")
                nc.sync.dma_start(out=wc[:], in_=w_d[c * P : (c + 1) * P, :])
                wt.append(wc)
            biasb = constp.tile([P, F_OUT], f32)
            nc.sync.dma_start(out=biasb[:], in_=bias_d[:, :])
            prelub = constp.tile([P, F_OUT], f32)
            nc.sync.dma_start(out=prelub[:], in_=prelu_d[:, :])

            # idx array resident in SBUF
            idx_all = disp.tile([P, T * 8], i16, name="idx_all")
            nc.sync.dma_start(out=idx_all[:], in_=idx16_d[:, :])

            dis_b = disp.tile([P, nw], f32)
            nc.sync.dma_start(out=dis_b[:], in_=degb_d[:, :])
            nc.scalar.activation(out=dis_b[:], in_=dis_b[:], func=ACT.Sqrt)
            nc.vector.reciprocal(out=dis_b[:], in_=dis_b[:])

            accw = accp.tile([P, nw * F_OUT], f32, name="accw")

            # chunk table: (sub-table, first tile, real tiles)
            chunks = []
            for s in range(NSUB):
                start, real, padded = segs[s]
                for ck0 in range(0, real, GCT):
                    chunks.append((s, start + ck0, min(real - ck0, GCT)))
            NCH = len(chunks)

            # ---------------- Phase B': g block = dis * (x @ W) ----------
            with nc.named_scope("phaseB"):
                chunks_b = list(range(0, BLK, XCH))
                LA = 2
                xtiles = {}

                def load_x(ci):
                    if ci >= len(chunks_b):
                        return
                    c0 = chunks_b[ci]
                    cl = min(XCH, BLK - c0)
                    xt = bxp.tile([P, 2 * XCH], bf16, tag="xt", name="xt")
                    nc.scalar.dma_start(
                        out=xt[:, : 2 * cl].rearrange("p (t c) -> p t c", t=2),
                        in_=x_blk[:, c0 : c0 + cl].rearrange(
                            "(t p) c -> p t c", p=P
                        ),
                    )
                    xtiles[ci] = xt

                for ci in range(LA):
                    load_x(ci)
                for ci, c0 in enumerate(chunks_b):
                    cl = min(XCH, BLK - c0)
                    load_x(ci + LA)
                    xt = xtiles.pop(ci)
                    gt = bgp.tile([P, GB * F_OUT], bf16, tag="bg")
                    jt0 = c0 // P
                    ntiles = cl // P
                    assert ntiles <= GB
                    for jj in range(ntiles):
                        j = jt0 + jj
                        ph = cpsp.tile([P, F_OUT], f32, tag="cps", name="cps")
                        for c in range(kchunks):
                            nc.tensor.matmul(
                                out=ph[:],
                                lhsT=xt[:, c * cl + jj * P : c * cl + (jj + 1) * P],
                                rhs=wt[c][:],
                                start=(c == 0),
                                stop=(c == kchunks - 1),
                            )
                        # g row = dis * h (Scalar engine), AG input
                        nc.scalar.mul(
                            gt[:, jj * F_OUT : (jj + 1) * F_OUT],
                            ph[:],
                            dis_b[:, j : j + 1],
                        )
                        # self-loop seed: accw = dis * h (DVE, f32)
                        nc.vector.tensor_scalar(
                            out=accw[:, j * F_OUT : (j + 1) * F_OUT],
                            in0=ph[:],
                            scalar1=dis_b[:, j : j + 1],
                            scalar2=None,
                            op0=AOT.mult,
                        )
                    # partition-major block write: row 128j+p at p*bt+j
                    nc.scalar.dma_start(
                        out=ag_in.rearrange("(p j) f -> p j f", p=P)[
                            :, jt0 : jt0 + ntiles, :
                        ],
                        in_=gt[:, : ntiles * F_OUT].rearrange(
                            "p (j f) -> p j f", f=F_OUT
                        ),
                    )

                nc.gpsimd.collective_compute(
                    "AllGather",
                    mybir.AluOpType.bypass,
                    replica_groups=[list(range(CORES))],
                    ins=[ag_in[:, :]],
                    outs=[ag_out[:, :]],
                )

            # ---------------- Phase C: gathers + scatter matmuls ---------
            with nc.named_scope("phaseC"):
                slast = [
                    max(s for s in range(NSUB) if (tsw[w][s] > 0 or s == 0))
                    for w in range(nw)
                ]

                def flush(w):
                    acc = cfp.tile([P, F_OUT], f32, tag="facc", name="facc")
                    nc.scalar.mul(
                        acc[:],
                        accw[:, w * F_OUT : (w + 1) * F_OUT],
                        dis_b[:, w : w + 1],
                    )
                    nc.vector.tensor_tensor(
                        out=acc[:], in0=acc[:], in1=biasb[:], op=AOT.add
                    )
                    neg = cfp.tile([P, F_OUT], f32, tag="fneg", name="fneg")
                    nc.vector.tensor_tensor(
                        out=neg[:], in0=acc[:], in1=prelub[:], op=AOT.mult
                    )
                    nc.vector.tensor_tensor(
                        out=accw[:, w * F_OUT : (w + 1) * F_OUT],
                        in0=acc[:],
                        in1=neg[:],
                        op=AOT.max,
                    )

                ci = 0
                for s in range(NSUB):
                    start, real, padded = segs[s]
                    tmap = []
                    bfirst = []
                    blast = []
                    for w in range(nw):
                        nt = tsw[w][s]
                        tmap += [w] * nt
                        bfirst += [True] + [False] * (nt - 1) if nt else []
                        blast += [False] * (nt - 1) + [True] if nt else []
                    cur = -1
                    oht = gch = None
                    ps = None
                    for tg in range(real):
                        ck = tg // GCT
                        if ck != cur:
                            cur = ck
                            s_c, t0, nreal = chunks[ci]
                            assert s_c == s and t0 == start + ck * GCT
                            gch = cgp.tile([P, GCT * F_OUT], bf16, tag="cg", name="cg")
                            nc.gpsimd.dma_gather(
                                out_ap=gch[:].rearrange("p (n e) -> p n e", e=F_OUT),
                                in_ap=ag_out[s * SUBR : (s + 1) * SUBR, :],
                                idxs_ap=idx_all[:, t0 * 8 : (t0 + GCT) * 8],
                                num_idxs=GCT * P,
                                num_idxs_reg=nreal * P,
                                elem_size=F_OUT,
                                single_packet=True,
                                queue_num=ci % 4,
                            )
                            oht = cohp.tile([P, GCT * P], fp8, tag="oh", name="oh")
                            nc.sync.dma_start(
                                out=oht[:, : nreal * P],
                                in_=oh_d[:, t0 * P : (t0 + nreal) * P],
                            )
                            ci += 1
                        ti = tg - cur * GCT
                        w = tmap[tg]
                        if bfirst[tg]:
                            ps = cpsp.tile([P, F_OUT], f32, tag="cps", name="cps")
                        nc.tensor.matmul(
                            out=ps[:],
                            lhsT=oht[:, ti * P : (ti + 1) * P],
                            rhs=gch[:, ti * F_OUT : (ti + 1) * F_OUT],
                            start=bfirst[tg],
                            stop=blast[tg],
                        )
                        if blast[tg]:
                            nc.vector.tensor_tensor(
                                out=accw[:, w * F_OUT : (w + 1) * F_OUT],
                                in0=accw[:, w * F_OUT : (w + 1) * F_OUT],
                                in1=ps[:],
                                op=AOT.add,
                            )
                            if s == slast[w]:
                                flush(w)
                assert ci == NCH
                nc.gpsimd.dma_start(
                    out=out_d.rearrange("(w p) f -> p w f", p=P),
                    in_=accw[:].rearrange("p (w f) -> p w f", f=F_OUT),
                )

    nc.compile()
    return nc


def _get_program(cfg, tsw, debug_outs=False):
    key = (tuple(sorted(cfg.items())), tsw, debug_outs)
    if key not in _prog_cache:
        _prog_cache[key] = build_program(cfg, tsw, debug_outs)
    return _prog_cache[key]


def make_in_maps(prep):
    return [
        {
            "x_blk": prep["x_blk"][k],
            "w": prep["w"],
            "bias_b": prep["bias_b"],
            "prelu_b": prep["prelu_b"],
            "deg_blk": prep["deg_blk"][k],
            "idx16": prep["idx16"][k],
            "oh8": prep["oh8"][k],
        }
        for k in range(CORES)
    ]


def kernel(x, edge_index, W, bias, prelu_a, cfg=None):
    from concourse import bass_utils

    cfg = cfg or FULL_CFG
    cfg = dict(cfg)
    prep = host_prep(x, edge_index, W, bias, prelu_a, cfg)
    nc = _get_program(cfg, prep["tsw"])
    res = bass_utils.run_bass_kernel_spmd(
        nc, make_in_maps(prep), core_ids=list(range(CORES))
    )
    N = cfg["N"]
    NT, NP, bt, BLK = _derived(cfg)
    outs = []
    for k in range(CORES):
        lo = k * BLK
        hi = min((k + 1) * BLK, N)
        outs.append(res.results[k]["out"][: hi - lo])
    return np.concatenate(outs, axis=0).astype(np.float32)


# revision 21
# speedup vs baseline: 1.7582x; 1.4846x over previous
"""GCNConv + PReLU on Trainium2, 8-core SPMD Bass/Tile kernel (v5).

Math (PyG GCNConv, add_self_loops=True, symmetric norm):
    h = x @ W
    deg[c] = (# edges with col == c) + 1          (self-loop)
    dis = rsqrt(deg)
    out[c] = dis[c] * ( sum_{e: col_e == c} dis[row_e] * h[row_e]
                        + dis[c] * h[c] )         (self-loop folded in)
             + bias
    z = max(out, prelu_a * out)                   (PReLU, 0 <= a <= 1)

Distribution (v3+): node rows split into 8 blocks of BLK=12544 rows;
each core computes g = dis*h for its block only, AllGather shares the
table; dest shard == source block so the self-loop term seeds the accw
accumulator during the same pass.

v5 phase C: per-edge dma_gather of source g rows from the DRAM table.
Trace analysis showed the binding constraint is the SDMA per-descriptor
processing wall (~100ns/desc/engine) when each 256B descriptor is its
own packet (single_packet=False). v5 uses single_packet=True with
1024-index chunks (64 descriptors per engine = the HW packet-coalescing
limit), which lets each engine stream its descriptors back-to-back.
"""

import math
import sys

for _p in ("/opt/trn_rl_repo",):
    if _p not in sys.path:
        sys.path.insert(0, _p)

import numpy as np
import ml_dtypes

P = 128
CORES = 8
NSUB = 4
GCT = 8  # tiles per dma_gather chunk (64 descs/engine: single-packet max)
XCH = 1024  # x columns per load
GB = 8  # g row-tiles staged per DMA write (= XCH // P)
GBUFS = 12  # gathered-chunk SBUF ring depth

FULL_CFG = dict(N=100000, F_IN=256, F_OUT=128, E=1600000)

_prog_cache = {}


def _derived(cfg):
    N = cfg["N"]
    NT = math.ceil(N / P)  # global row-tiles
    NT = -(-NT // CORES) * CORES  # pad so blocks split evenly
    NP = NT * P
    bt = NT // CORES  # row-tiles per block
    BLK = bt * P  # rows per block (dest shard == source block)
    # sub-table s = blocks 2s, 2s+1 of the AllGather output; row count
    # per sub-table must stay under the int16 gather-index limit.
    assert 2 * BLK <= 32767
    return NT, NP, bt, BLK


def _schedule(tsw, nw):
    """Segment layout: one contiguous tile run per sub-table s
    (w-ascending within), padded to a GCT multiple. Returns per-(w,s)
    tile base offsets, segment table, and total padded tile count T."""
    tile_base = np.zeros((nw, NSUB), np.int64)
    segs = []  # (start, real, padded) per s
    pos = 0
    for s in range(NSUB):
        start = pos
        for w in range(nw):
            tile_base[w, s] = pos
            pos += tsw[w][s][0]
        real = pos - start
        padded = -(-real // GCT) * GCT
        segs.append((start, real, padded))
        pos = start + padded
    return tile_base, segs, pos


def host_prep(x, edge_index, W, bias, prelu_a, cfg):
    """Index/layout prep: integer degree histogram, shard + sort edges by
    (dest window, source sub-table), fixed 128-edge tiles, int16 wrapped
    gather indices (partition-major g layout), fp8 one-hot tiles, and
    bf16 cast/transpose of x. All float math on x/W stays on device."""
    N, F_IN, F_OUT = cfg["N"], cfg["F_IN"], cfg["F_OUT"]
    NT, NP, bt, BLK = _derived(cfg)
    nw = bt  # dest windows per core == block row-tiles

    x = np.asarray(x, np.float32)
    W = np.asarray(W, np.float32)
    bias = np.asarray(bias, np.float32)
    prelu_a = np.asarray(prelu_a, np.float32)
    ei = np.asarray(edge_index)

    rows_all = np.asarray(ei[0]).astype(np.int64)
    cols_all = np.asarray(ei[1]).astype(np.int64)

    # integer degree histogram (+1 self-loop); pad rows get deg=1
    deg = np.bincount(cols_all, minlength=N).astype(np.float32) + 1.0
    deg_pad = np.ones(NP, np.float32)
    deg_pad[:N] = deg
    # per-core block layout [CORES, P, bt]: deg_blk[k, p, j] = deg of
    # global row k*BLK + j*P + p
    deg_blk = np.ascontiguousarray(
        deg_pad.reshape(CORES, bt, P).transpose(0, 2, 1)
    )

    order = np.argsort(cols_all, kind="stable")
    rs = rows_all[order]
    cs = cols_all[order]
    bounds = np.searchsorted(cs, np.arange(CORES + 1) * BLK)

    # source row r -> gather position: block k = j//bt (j = r//P), then
    # partition-major within block: pos = k*BLK + (r%P)*bt + j%bt.
    # Sub-table s = k//2, in-table index = pos - s*2*BLK.
    cnts = np.zeros((CORES, nw, NSUB), np.int64)
    per_core = []
    for k in range(CORES):
        seg = slice(bounds[k], bounds[k + 1])
        local = cs[seg] - k * BLK
        w_arr = local // P
        r_arr = rs[seg]
        j_arr = r_arr // P
        s_arr = j_arr // (2 * bt)
        key = w_arr * NSUB + s_arr
        o2 = np.argsort(key, kind="stable")
        cnts[k] = np.bincount(key, minlength=nw * NSUB).reshape(nw, NSUB)
        per_core.append((local[o2], w_arr[o2], s_arr[o2], r_arr[o2], key[o2]))

    bmax = cnts.max(axis=0)  # [nw, NSUB] exact per-bucket slot counts
    tsw = -(-bmax // P)
    tsw[:, 0] = np.maximum(tsw[:, 0], 1)  # every window needs >= 1 matmul
    # gather exactly ceil16(bmax) slots per bucket (descriptor count is
    # the phase-C floor); the rest of the bucket's padded tiles is never
    # written and the one-hot zeros it out of the matmul
    bm16 = np.minimum(-(-bmax // 16) * 16, tsw * P)
    tsw_l = tuple(
        tuple((int(t), int(b)) for t, b in zip(trow, brow))
        for trow, brow in zip(tsw, bm16)
    )
    tile_base, segs, T = _schedule(tsw_l, nw)

    idx16 = np.zeros((CORES, 16, T * 8), np.int16)
    oh8 = np.zeros((CORES, P, T * P), ml_dtypes.float8_e4m3)
    for k in range(CORES):
        local, w_arr, s_arr, r_arr, key = per_core[k]
        cnt_flat = cnts[k].reshape(-1)
        gstart = np.concatenate([[0], np.cumsum(cnt_flat)])
        within = np.arange(local.size) - gstart[key]
        slot = tile_base[w_arr, s_arr] * P + within
        p_arr = slot % P
        t_arr = slot // P
        # gather position: partition-major within block, block-major in s
        kb = (r_arr // P // bt) % 2
        pos = kb * BLK + (r_arr % P) * bt + (r_arr // P) % bt
        idx16[k, p_arr % 16, t_arr * 8 + p_arr // 16] = pos.astype(np.int16)
        oh8[k, p_arr, t_arr * P + (local - w_arr * P)] = 1.0
        # trailing pad tiles of each segment: idx = -1 (descriptors skipped)
        for start, real, padded in segs:
            if padded > real:
                idx16[k, :, (start + real) * 8 : (start + padded) * 8] = -1
    idx16_rep = np.ascontiguousarray(np.tile(idx16, (1, P // 16, 1)))

    xp = np.zeros((NP, F_IN), np.float32)
    xp[:N] = x
    x_t = np.ascontiguousarray(xp.T.astype(ml_dtypes.bfloat16))  # [F_IN, NP]
    # per-core block slice of x (transposed): [CORES, F_IN, BLK]
    x_blk = np.ascontiguousarray(x_t.reshape(F_IN, CORES, BLK).transpose(1, 0, 2))

    return dict(
        tsw=tsw_l,
        T=T,
        x_blk=x_blk,
        w=np.ascontiguousarray(W.astype(ml_dtypes.bfloat16)),
        bias_b=np.ascontiguousarray(np.tile(bias[None, :], (P, 1))),
        prelu_b=np.ascontiguousarray(np.tile(prelu_a[None, :], (P, 1))),
        deg_blk=deg_blk,
        idx16=idx16_rep,
        oh8=oh8,
    )


def build_program(cfg, tsw, debug_outs=False):
    import concourse.bass as bass
    import concourse.bacc as bacc
    import concourse.mybir as mybir
    import concourse.tile as tile
    from concourse.bass import ds

    f32 = mybir.dt.float32
    bf16 = mybir.dt.bfloat16
    fp8 = mybir.dt.float8e4
    i16 = mybir.dt.int16
    AOT = mybir.AluOpType
    ACT = mybir.ActivationFunctionType

    N, F_IN, F_OUT = cfg["N"], cfg["F_IN"], cfg["F_OUT"]
    NT, NP, bt, BLK = _derived(cfg)
    nw = bt
    kchunks = F_IN // P
    tile_base, segs, T = _schedule(tsw, nw)

    nc = bacc.Bacc(
        "TRN2",
        target_bir_lowering=False,
        debug=False,
        num_devices=CORES,
        num_swdge_queues=4,
    )

    x_blk = nc.dram_tensor("x_blk", [F_IN, BLK], bf16, kind="ExternalInput")
    w_d = nc.dram_tensor("w", [F_IN, F_OUT], bf16, kind="ExternalInput")
    bias_d = nc.dram_tensor("bias_b", [P, F_OUT], f32, kind="ExternalInput")
    prelu_d = nc.dram_tensor("prelu_b", [P, F_OUT], f32, kind="ExternalInput")
    degb_d = nc.dram_tensor("deg_blk", [P, nw], f32, kind="ExternalInput")
    idx16_d = nc.dram_tensor("idx16", [P, T * 8], i16, kind="ExternalInput")
    oh_d = nc.dram_tensor("oh8", [P, T * P], fp8, kind="ExternalInput")
    out_d = nc.dram_tensor("out", [BLK, F_OUT], f32, kind="ExternalOutput")

    # AllGather: per-core g block (partition-major) -> full table, of
    # which the 4 gather sub-tables are row-range views.
    ag_in = nc.dram_tensor("ag_in", [BLK, F_OUT], bf16)
    ag_out = nc.dram_tensor(
        "ag_out", [CORES * BLK, F_OUT], bf16, addr_space="Shared"
    )
    SUBR = 2 * BLK  # rows per sub-table view

    with tile.TileContext(nc, pool_alloc_mode="queue") as tc:
        with (
            tc.tile_pool(name="const", bufs=1) as constp,
            tc.tile_pool(name="dis", bufs=1) as disp,
            tc.tile_pool(name="acc", bufs=1) as accp,
            tc.tile_pool(name="c_oh", bufs=6) as cohp,
            tc.tile_pool(name="c_g", bufs=GBUFS) as cgp,
            tc.tile_pool(name="c_ps", bufs=8, space="PSUM") as cpsp,
            tc.tile_pool(name="c_f", bufs=4) as cfp,
            tc.tile_pool(name="b_x", bufs=3) as bxp,
            tc.tile_pool(name="b_g", bufs=4) as bgp,
        ):
            wt = []
            for c in range(kchunks):
                wc = constp.tile([P, F_OUT], bf16, tag=f"wc{c}")
                nc.sync.dma_start(out=wc[:], in_=w_d[c * P : (c + 1) * P, :])
                wt.append(wc)
            biasb = constp.tile([P, F_OUT], f32)
            nc.sync.dma_start(out=biasb[:], in_=bias_d[:, :])
            prelub = constp.tile([P, F_OUT], f32)
            nc.sync.dma_start(out=prelub[:], in_=prelu_d[:, :])

            # idx array resident in SBUF
            idx_all = disp.tile([P, T * 8], i16, name="idx_all")
            nc.sync.dma_start(out=idx_all[:], in_=idx16_d[:, :])

            dis_b = disp.tile([P, nw], f32)
            nc.sync.dma_start(out=dis_b[:], in_=degb_d[:, :])
            nc.scalar.activation(out=dis_b[:], in_=dis_b[:], func=ACT.Sqrt)
            nc.vector.reciprocal(out=dis_b[:], in_=dis_b[:])

            accw = accp.tile([P, nw * F_OUT], f32, name="accw")

            GMAX = max(tsw[w][s][0] for w in range(nw) for s in range(NSUB))

            # ---------------- Phase B': g block = dis * (x @ W) ----------
            with nc.named_scope("phaseB"):
                chunks_b = list(range(0, BLK, XCH))
                LA = 2
                xtiles = {}

                def load_x(ci):
                    if ci >= len(chunks_b):
                        return
                    c0 = chunks_b[ci]
                    cl = min(XCH, BLK - c0)
                    xt = bxp.tile([P, 2 * XCH], bf16, tag="xt", name="xt")
                    nc.scalar.dma_start(
                        out=xt[:, : 2 * cl].rearrange("p (t c) -> p t c", t=2),
                        in_=x_blk[:, c0 : c0 + cl].rearrange(
                            "(t p) c -> p t c", p=P
                        ),
                    )
                    xtiles[ci] = xt

                for ci in range(LA):
                    load_x(ci)
                for ci, c0 in enumerate(chunks_b):
                    cl = min(XCH, BLK - c0)
                    load_x(ci + LA)
                    xt = xtiles.pop(ci)
                    gt = bgp.tile([P, GB * F_OUT], bf16, tag="bg")
                    jt0 = c0 // P
                    ntiles = cl // P
                    assert ntiles <= GB
                    for jj in range(ntiles):
                        j = jt0 + jj
                        ph = cpsp.tile([P, F_OUT], f32, tag="cps", name="cps")
                        for c in range(kchunks):
                            nc.tensor.matmul(
                                out=ph[:],
                                lhsT=xt[:, c * cl + jj * P : c * cl + (jj + 1) * P],
                                rhs=wt[c][:],
                                start=(c == 0),
                                stop=(c == kchunks - 1),
                            )
                        # g row = dis * h (Scalar engine), AG input
                        nc.scalar.mul(
                            gt[:, jj * F_OUT : (jj + 1) * F_OUT],
                            ph[:],
                            dis_b[:, j : j + 1],
                        )
                        # self-loop seed: accw = dis * h (DVE, f32)
                        nc.vector.tensor_scalar(
                            out=accw[:, j * F_OUT : (j + 1) * F_OUT],
                            in0=ph[:],
                            scalar1=dis_b[:, j : j + 1],
                            scalar2=None,
                            op0=AOT.mult,
                        )
                    # partition-major block write: row 128j+p at p*bt+j
                    nc.scalar.dma_start(
                        out=ag_in.rearrange("(p j) f -> p j f", p=P)[
                            :, jt0 : jt0 + ntiles, :
                        ],
                        in_=gt[:, : ntiles * F_OUT].rearrange(
                            "p (j f) -> p j f", f=F_OUT
                        ),
                    )

                nc.gpsimd.collective_compute(
                    "AllGather",
                    mybir.AluOpType.bypass,
                    replica_groups=[list(range(CORES))],
                    ins=[ag_in[:, :]],
                    outs=[ag_out[:, :]],
                )

            # ---------------- Phase C: per-bucket gathers + scatter ------
            # One dma_gather per (window, sub-table) bucket with the exact
            # (max-over-cores, ceil-16) slot count: descriptor count is the
            # phase-C floor (~160 descs/us/core HBM random-read wall), so
            # pad descriptors are what we cut. Bucket tiles beyond the
            # gathered count stay at their ring-buffer contents (memset
            # once below); the one-hot zeros them out of the matmul.
            with nc.named_scope("phaseC"):
                slast = [
                    max(s for s in range(NSUB) if (tsw[w][s][1] > 0 or s == 0))
                    for w in range(nw)
                ]


                def flush(w):
                    acc = cfp.tile([P, F_OUT], f32, tag="facc", name="facc")
                    nc.scalar.mul(
                        acc[:],
                        accw[:, w * F_OUT : (w + 1) * F_OUT],
                        dis_b[:, w : w + 1],
                    )
                    nc.vector.tensor_tensor(
                        out=acc[:], in0=acc[:], in1=biasb[:], op=AOT.add
                    )
                    neg = cfp.tile([P, F_OUT], f32, tag="fneg", name="fneg")
                    nc.vector.tensor_tensor(
                        out=neg[:], in0=acc[:], in1=prelub[:], op=AOT.mult
                    )
                    nc.vector.tensor_tensor(
                        out=accw[:, w * F_OUT : (w + 1) * F_OUT],
                        in0=acc[:],
                        in1=neg[:],
                        op=AOT.max,
                    )

                bi = 0
                for s in range(NSUB):
                    for w in range(nw):
                        nt, bm = tsw[w][s]
                        if nt == 0:
                            continue
                        t0 = int(tile_base[w][s])
                        gch = cgp.tile(
                            [P, GMAX * F_OUT], bf16, tag="cg", name="cg"
                        )
                        if bm < nt * P:
                            # gather leaves slots [bm, nt*128) untouched;
                            # zero the partial tile so the (oh=0) matmul
                            # never reads poison
                            nc.vector.memset(
                                gch[:, (nt - 1) * F_OUT : nt * F_OUT], 0.0
                            )
                        if bm > 0:
                            nc.gpsimd.dma_gather(
                                out_ap=gch[:, : nt * F_OUT].rearrange(
                                    "p (n e) -> p n e", e=F_OUT
                                ),
                                in_ap=ag_out[s * SUBR : (s + 1) * SUBR, :],
                                idxs_ap=idx_all[
                                    :, t0 * 8 : t0 * 8 + -(-bm // 16)
                                ],
                                num_idxs=bm,
                                num_idxs_reg=bm,
                                elem_size=F_OUT,
                                single_packet=bool(bm <= 1024),
                                queue_num=bi % 4,
                            )
                        oht = cohp.tile([P, GMAX * P], fp8, tag="oh", name="oh")
                        nc.sync.dma_start(
                            out=oht[:, : nt * P],
                            in_=oh_d[:, t0 * P : (t0 + nt) * P],
                        )
                        bi += 1
                        ps = cpsp.tile([P, F_OUT], f32, tag="cps", name="cps")
                        for ti in range(nt):
                            nc.tensor.matmul(
                                out=ps[:],
                                lhsT=oht[:, ti * P : (ti + 1) * P],
                                rhs=gch[:, ti * F_OUT : (ti + 1) * F_OUT],
                                start=(ti == 0),
                                stop=(ti == nt - 1),
                            )
                        nc.vector.tensor_tensor(
                            out=accw[:, w * F_OUT : (w + 1) * F_OUT],
                            in0=accw[:, w * F_OUT : (w + 1) * F_OUT],
                            in1=ps[:],
                            op=AOT.add,
                        )
                        if s == slast[w]:
                            flush(w)
                nc.gpsimd.dma_start(
                    out=out_d.rearrange("(w p) f -> p w f", p=P),
                    in_=accw[:].rearrange("p (w f) -> p w f", f=F_OUT),
                )

    nc.compile()
    return nc


def _get_program(cfg, tsw, debug_outs=False):
    key = (tuple(sorted(cfg.items())), tsw, debug_outs)
    if key not in _prog_cache:
        _prog_cache[key] = build_program(cfg, tsw, debug_outs)
    return _prog_cache[key]


def make_in_maps(prep):
    return [
        {
            "x_blk": prep["x_blk"][k],
            "w": prep["w"],
            "bias_b": prep["bias_b"],
            "prelu_b": prep["prelu_b"],
            "deg_blk": prep["deg_blk"][k],
            "idx16": prep["idx16"][k],
            "oh8": prep["oh8"][k],
        }
        for k in range(CORES)
    ]


def kernel(x, edge_index, W, bias, prelu_a, cfg=None):
    from concourse import bass_utils

    cfg = cfg or FULL_CFG
    cfg = dict(cfg)
    prep = host_prep(x, edge_index, W, bias, prelu_a, cfg)
    nc = _get_program(cfg, prep["tsw"])
    res = bass_utils.run_bass_kernel_spmd(
        nc, make_in_maps(prep), core_ids=list(range(CORES))
    )
    N = cfg["N"]
    NT, NP, bt, BLK = _derived(cfg)
    outs = []
    for k in range(CORES):
        lo = k * BLK
        hi = min((k + 1) * BLK, N)
        outs.append(res.results[k]["out"][: hi - lo])
    return np.concatenate(outs, axis=0).astype(np.float32)
